# revision 27
# baseline (speedup 1.0000x reference)
"""Fused multi-head attention forward (B=2, S=2048, SIZE=1024, H=16) on 8
Trainium2 NeuronCores.

Sharding: 2-way data parallel over batch x 4-way tensor parallel over heads
(Megatron style). Each core computes 4 heads of one batch element end-to-end
(QKV projections for its 256-dim slice, attention, and a partial output
projection); the host sums the 4 partials per batch and adds the output
bias. The value-projection bias drops out of attention algebraically
(softmax rows sum to 1), so the host folds `bv @ Wo.T` into that same
constant row.

On-device layouts avoid all transposes:
  qhT/khT [dim, s]  <- host-transposed qT/kT as matmul rhs, WqT/WkT as lhsT
  vh      [s, dim]  <- vT tiles as lhsT, WvT as rhs (natural layout)
  scoresT [k, q]    <- khT as lhsT, qhT as rhs (c = 64)
  ctxT    [dim, q]  <- vh as lhsT, exp(scoresT) as rhs; a ones column fused
                       into vh makes row 64 of the accumulator the softmax
                       denominator for free
  out     [s, o]    <- ctxT as lhsT, WoT as rhs (c = 128, heads paired)

Attention processes head PAIRS with a q-range split (head h0 sweeps one half
of q while h1 sweeps the other, then they swap): the two heads' c=64 score
matmuls sit in adjacent row groups (partitions 0-63 / 64-127) so the PE
runs them as concurrent streams, and the PSUM footprint stays at 8 banks
(2x scores [128,1024] + 4x ctx [65,512]).

Softmax denominators roundtrip through DRAM twice: once spread over 64
partitions for a fast DVE reciprocal, once broadcast for the normalize.
Matmul operands are bf16 (fp32 PSUM accumulation); the attention scale
1/sqrt(64) is folded into Wq/bq on host.
"""

import numpy as np
import ml_dtypes

import concourse.bass as bass
import concourse.tile as tile
from concourse import bacc, mybir
from concourse.bass_utils import run_bass_kernel_spmd

B, S, SIZE, H, D = 2, 2048, 1024, 16, 64
NCORES = 8
HGROUPS = 4                # tensor-parallel head groups
H_LOC = H // HGROUPS       # 4 heads per core
D_LOC = H_LOC * D          # 256 projection dims per core
MT = D_LOC // 128          # 2 head-pairs per core
ET = SIZE // 128           # 8 contraction tiles for projections
ST = S // 128              # 16 sequence tiles of 128
QT = S // 512              # 4 sequence chunks of 512
KT = S // 128              # 16 key tiles

_NC = None


def build():
    global _NC
    if _NC is not None:
        return _NC
    f32, bf16 = mybir.dt.float32, mybir.dt.bfloat16
    Exp = mybir.ActivationFunctionType.Exp

    nc = bacc.Bacc("TRN2", target_bir_lowering=False, debug=False)
    qT_d = nc.dram_tensor("qT", [SIZE, S], bf16, kind="ExternalInput").ap()
    kT_d = nc.dram_tensor("kT", [SIZE, S], bf16, kind="ExternalInput").ap()
    vT_d = nc.dram_tensor("vT", [SIZE, S], bf16, kind="ExternalInput").ap()
    WqT_d = nc.dram_tensor("WqT", [SIZE, D_LOC], bf16, kind="ExternalInput").ap()
    WkT_d = nc.dram_tensor("WkT", [SIZE, D_LOC], bf16, kind="ExternalInput").ap()
    WvT_d = nc.dram_tensor("WvT", [SIZE, D_LOC], bf16, kind="ExternalInput").ap()
    WoT_d = nc.dram_tensor("WoT", [D_LOC, SIZE], bf16, kind="ExternalInput").ap()
    bq_d = nc.dram_tensor("bq", [D_LOC], f32, kind="ExternalInput").ap()
    bk_d = nc.dram_tensor("bk", [D_LOC], f32, kind="ExternalInput").ap()
    out_d = nc.dram_tensor("out", [S, SIZE], f32, kind="ExternalOutput").ap()

    qTt = qT_d.rearrange("(et p) s -> p et s", p=128)
    kTt = kT_d.rearrange("(et p) s -> p et s", p=128)
    vTt = vT_d.rearrange("(et p) s -> p et s", p=128)

    with tile.TileContext(nc) as tc:
        with tc.tile_pool(name="persist", bufs=1) as persist:
            wq_sb = persist.tile([128, ET, D_LOC], bf16)
            wk_sb = persist.tile([128, ET, D_LOC], bf16)
            wv_sb = persist.tile([128, ET, D_LOC], bf16)
            nc.sync.dma_start(wv_sb[:], WvT_d.rearrange("(et p) m -> p et m", p=128))
            nc.sync.dma_start(wq_sb[:], WqT_d.rearrange("(et p) m -> p et m", p=128))
            nc.sync.dma_start(wk_sb[:], WkT_d.rearrange("(et p) m -> p et m", p=128))
            wo_sb = persist.tile([128, MT, SIZE], bf16)
            nc.sync.dma_start(wo_sb[:], WoT_d.rearrange("(hp p) o -> p hp o", p=128))
            bq_sb = persist.tile([128, MT], f32)
            bk_sb = persist.tile([128, MT], f32)
            nc.sync.dma_start(bq_sb[:], bq_d.rearrange("(mt p) -> p mt", p=128))
            nc.sync.dma_start(bk_sb[:], bk_d.rearrange("(mt p) -> p mt", p=128))

            qh_sb = persist.tile([128, MT, S], bf16)   # [dim within pair, pair, s]
            kh_sb = persist.tile([128, MT, S], bf16)
            vh_sb = persist.tile([128, ST, H_LOC, D], bf16)  # [s%128, s//128, head, d]
            ctx_sb = persist.tile([128, MT, S], bf16)  # normalized ctxT, head pairs on partitions
            ones_f32 = persist.tile([128, 1], f32)
            nc.vector.memset(ones_f32[:], 1.0)
            ones_bf = persist.tile([128, 1], bf16)
            nc.vector.tensor_copy(ones_bf[:], ones_f32[:])

            # ---------- phase A: projections, one row-DMA per contraction tile ----------
            with (
                tc.tile_pool(name="xinV", bufs=3) as xinV,
                tc.tile_pool(name="psV", bufs=1, space="PSUM") as psV,
            ):
                # V first, in 2 passes of 8 sequence tiles (8 PSUM banks each)
                for vpass in range(2):
                    psv = [psV.tile([128, D_LOC], f32, tag=f"pv{i}", name=f"pv{i}")
                           for i in range(8)]
                    for et in range(ET):
                        vrow = xinV.tile([128, 1024], bf16, tag="vrow")
                        nc.sync.dma_start(
                            vrow[:], vTt[:, et, vpass * 1024:(vpass + 1) * 1024])
                        for i in range(8):
                            nc.tensor.matmul(
                                psv[i][:], vrow[:, i * 128:(i + 1) * 128], wv_sb[:, et, :],
                                start=(et == 0), stop=(et == ET - 1))
                    for i in range(8):
                        st = vpass * 8 + i
                        nc.vector.tensor_copy(
                            vh_sb[:, st, :, :],
                            psv[i].rearrange("p (h d) -> p h d", h=H_LOC))
            with (
                tc.tile_pool(name="xinQK", bufs=3) as xinQK,
                tc.tile_pool(name="psQK", bufs=1, space="PSUM") as psQK,
            ):
                for src, wsb, bsb, dst in (
                    (qTt, wq_sb, bq_sb, qh_sb),
                    (kTt, wk_sb, bk_sb, kh_sb),
                ):
                    pss = [psQK.tile([128, 512], f32, tag=f"pa{i}", name=f"pa{i}")
                           for i in range(8)]
                    for et in range(ET):
                        xrow = xinQK.tile([128, S], bf16, tag="xrow")
                        nc.sync.dma_start(xrow[:], src[:, et, :])
                        for nt in range(QT):
                            for mt in range(MT):
                                nc.tensor.matmul(
                                    pss[nt * MT + mt][:],
                                    wsb[:, et, mt * 128:(mt + 1) * 128],
                                    xrow[:, nt * 512:(nt + 1) * 512],
                                    start=(et == 0), stop=(et == ET - 1))
                    for nt in range(QT):
                        for mt in range(MT):
                            nc.vector.tensor_scalar_add(
                                dst[:, mt, nt * 512:(nt + 1) * 512],
                                pss[nt * MT + mt][:], bsb[:, mt:mt + 1])

            # ---------- phase B/C: attention, head pairs with q-split sweeps ----------
            with (
                tc.tile_pool(name="psS", bufs=1, space="PSUM") as psS,
                tc.tile_pool(name="psC", bufs=2, space="PSUM") as psC,
                tc.tile_pool(name="esb", bufs=4) as esb,
                tc.tile_pool(name="smalls", bufs=2) as smalls,
                tc.tile_pool(name="dscr", bufs=2, space="DRAM") as dscr,
            ):
                for pr in range(MT):
                    heads = (2 * pr, 2 * pr + 1)       # partition offsets 0, 64
                    for sw in range(2):
                        q0s = sw * 1024                # both heads sweep this q-half
                        # ctx col-packed: head 0 -> rows 0-63, head 1 -> rows
                        # 64-127 of one full-width accumulator per q-chunk
                        cpair = [psC.tile([128, 512], f32, tag=f"c{j}", name=f"c{j}")
                                 for j in range(2)]
                        # bf16 running sums are safe: only the 128-partition sum
                        # is used, so per-element rounding averages out; 4
                        # interleaved accumulators bound the sequential depth
                        eacc = [[smalls.tile([128, 1024], bf16, tag=f"eacc{hi}{a}",
                                             name=f"eacc{hi}{a}")
                                 for a in range(4)]
                                for hi in range(2)]
                        for kt in range(KT):
                            # both heads' scores share one PSUM tile so a single
                            # exp instruction covers them (halves ACT overhead)
                            sc = psS.tile([128, 2 * 1024], f32, tag="sc")
                            # interleave the two heads' matmuls: row groups 0-63 /
                            # 64-127 stream concurrently through the PE
                            for j in range(2):
                                for hi in range(2):
                                    po = hi * D
                                    q0 = q0s + j * 512
                                    c0 = hi * 1024 + j * 512
                                    nc.tensor.matmul(
                                        sc[:, c0:c0 + 512],
                                        kh_sb[po:po + D, pr, kt * 128:(kt + 1) * 128],
                                        qh_sb[po:po + D, pr, q0:q0 + 512],
                                        start=True, stop=True)
                            e_sb = esb.tile([128, 2 * 1024], bf16, tag="e")
                            nc.scalar.activation(e_sb[:], sc[:], Exp)
                            # softmax denominators: accumulate exp tiles on DVE
                            for hi in range(2):
                                a = kt % 4
                                esl = e_sb[:, hi * 1024:(hi + 1) * 1024]
                                if kt < 4:
                                    nc.vector.tensor_copy(eacc[hi][a][:], esl)
                                else:
                                    nc.vector.tensor_add(
                                        eacc[hi][a][:], eacc[hi][a][:], esl)
                            for j in range(2):
                                for hi in range(2):
                                    nc.tensor.matmul(
                                        cpair[j][hi * D:(hi + 1) * D, :],
                                        vh_sb[:, kt, heads[hi], :],
                                        e_sb[:, hi * 1024 + j * 512:
                                             hi * 1024 + (j + 1) * 512],
                                        start=(kt == 0), stop=(kt == KT - 1))
                        # evacuate accumulators (releases ctx PSUM banks), reduce
                        # eacc over partitions with tiny M=1 matmuls, reciprocal
                        # via the 64-partition DRAM spread, broadcast, multiply
                        cu = smalls.tile([128, 1024], f32, tag="cu")
                        for j in range(2):
                            nc.vector.tensor_copy(
                                cu[:, j * 512:(j + 1) * 512], cpair[j][:])
                        for hi in range(2):
                            nc.vector.tensor_add(
                                eacc[hi][0][:], eacc[hi][0][:], eacc[hi][1][:])
                            nc.vector.tensor_add(
                                eacc[hi][2][:], eacc[hi][2][:], eacc[hi][3][:])
                            nc.vector.tensor_add(
                                eacc[hi][0][:], eacc[hi][0][:], eacc[hi][2][:])
                        sums_sb = smalls.tile([1, 2048], f32, tag="sums")
                        for hi in range(2):
                            for j in range(2):
                                sp = psC.tile([1, 512], f32, tag=f"c{j}",
                                              name=f"sp{j}")
                                nc.tensor.matmul(
                                    sp[:], ones_bf[:],
                                    eacc[hi][0][:, j * 512:(j + 1) * 512],
                                    start=True, stop=True)
                                nc.vector.tensor_copy(
                                    sums_sb[:, hi * 1024 + j * 512:
                                            hi * 1024 + (j + 1) * 512], sp[:])
                        scr = dscr.tile([2048], f32, tag="scr")
                        nc.sync.dma_start(
                            scr[:].rearrange("(a b) -> a b", a=1), sums_sb[:])
                        spread = smalls.tile([64, 32], f32, tag="spread")
                        nc.sync.dma_start(
                            spread[:], scr[:].rearrange("(p j) -> p j", p=64))
                        spread_r = smalls.tile([64, 32], f32, tag="spreadr")
                        nc.vector.reciprocal(spread_r[:], spread[:])
                        scr2 = dscr.tile([2048], f32, tag="scr2")
                        nc.sync.dma_start(
                            scr2[:].rearrange("(p j) -> p j", p=64), spread_r[:])
                        brec = smalls.tile([128, 1024], f32, tag="brec")
                        for hi in range(2):
                            part = scr2[hi * 1024:(hi + 1) * 1024]
                            nc.sync.dma_start(
                                brec[hi * D:(hi + 1) * D, :],
                                bass.AP(tensor=part.tensor, offset=part.offset,
                                        ap=[[0, D]] + list(part.ap)))
                        for hi in range(2):
                            for j in range(2):
                                q0 = q0s + j * 512
                                rows = slice(hi * D, (hi + 1) * D)
                                nc.vector.tensor_mul(
                                    ctx_sb[rows, pr, q0:q0 + 512],
                                    cu[rows, j * 512:(j + 1) * 512],
                                    brec[rows, j * 512:(j + 1) * 512])

            # ---------- phase D: output projection (partial over local dims) ----------
            with (
                tc.tile_pool(name="psD", bufs=4, space="PSUM") as psD,
                tc.tile_pool(name="osb", bufs=4) as osb,
            ):
                for st in range(ST):
                    for ot in range(SIZE // 512):
                        pso = psD.tile([128, 512], f32, tag="po")
                        for hp in range(MT):
                            nc.tensor.matmul(
                                pso[:],
                                ctx_sb[:, hp, st * 128:(st + 1) * 128],
                                wo_sb[:, hp, ot * 512:(ot + 1) * 512],
                                start=(hp == 0), stop=(hp == MT - 1))
                        o_sb = osb.tile([128, 512], f32, tag="o")
                        if (st * 2 + ot) % 2 == 0:
                            nc.vector.tensor_copy(o_sb[:], pso[:])
                        else:
                            nc.scalar.copy(o_sb[:], pso[:])
                        nc.sync.dma_start(
                            out_d[st * 128:(st + 1) * 128, ot * 512:(ot + 1) * 512],
                            o_sb[:])

    nc.compile()
    _NC = nc
    return nc


def prepare_in_maps(inputs):
    q, k, v = inputs["q"], inputs["k"], inputs["v"]
    Wq, bq = inputs["Wq"], inputs["bq"]
    Wk, bk = inputs["Wk"], inputs["bk"]
    Wv = inputs["Wv"]
    Wo = inputs["Wo"]
    sc = np.float32(1.0 / np.sqrt(D))

    f32, bf = np.float32, ml_dtypes.bfloat16
    qT = [q[b].T.astype(bf) for b in range(B)]
    kT = [k[b].T.astype(bf) for b in range(B)]
    vT = [v[b].T.astype(bf) for b in range(B)]
    WqTs = (Wq.T * sc).astype(bf)   # scale folded into Wq
    WkT = Wk.T.astype(bf)
    WvT = Wv.T.astype(bf)
    WoT = Wo.T.astype(bf)           # [c, o]
    bqs = (bq * sc).astype(f32)

    in_maps = []
    for core in range(NCORES):
        b, hg = divmod(core, HGROUPS)
        sl = slice(hg * D_LOC, (hg + 1) * D_LOC)
        in_maps.append({
            "qT": qT[b], "kT": kT[b], "vT": vT[b],
            "WqT": np.ascontiguousarray(WqTs[:, sl]),
            "WkT": np.ascontiguousarray(WkT[:, sl]),
            "WvT": np.ascontiguousarray(WvT[:, sl]),
            "WoT": np.ascontiguousarray(WoT[sl, :]),
            "bq": np.ascontiguousarray(bqs[sl]),
            "bk": np.ascontiguousarray(bk[sl].astype(f32)),
        })
    return in_maps


def gather(results, inputs):
    # host epilogue: sum the 4 tensor-parallel partials per batch and add the
    # constant row bv @ Wo.T + bo (the value bias commutes through softmax)
    const = (inputs["bv"].astype(np.float64) @ inputs["Wo"].astype(np.float64).T
             + inputs["bo"].astype(np.float64)).astype(np.float32)
    full = np.empty((B, S, SIZE), np.float32)
    for b in range(B):
        acc = results[b * HGROUPS]["out"].astype(np.float32).copy()
        for hg in range(1, HGROUPS):
            acc += results[b * HGROUPS + hg]["out"]
        full[b] = acc + const[None, :]
    return full


def kernel(**inputs):
    nc = build()
    in_maps = prepare_in_maps(inputs)
    res = run_bass_kernel_spmd(nc, in_maps, core_ids=list(range(NCORES)), trace=False)
    return gather(res.results, inputs)


# revision 28
# speedup vs baseline: 1.2713x; 1.2713x over previous
"""Fused multi-head attention forward (B=2, S=2048, SIZE=1024, H=16) on 8
Trainium2 NeuronCores.

Sharding: 2-way data parallel over batch x 4-way tensor parallel over heads
(Megatron style). Each core computes 4 heads of one batch element end-to-end
(QKV projections for its 256-dim slice, attention, and a partial output
projection); the host sums the 4 partials per batch and adds the output
bias. The value-projection bias drops out of attention algebraically
(softmax rows sum to 1), so the host folds `bv @ Wo.T` into that same
constant row.

On-device layouts avoid all transposes:
  qhT/khT [dim, s]  <- host-transposed qT/kT as matmul rhs, WqT/WkT as lhsT
  vh      [s, dim]  <- vT tiles as lhsT, WvT as rhs (natural layout)
  scoresT [k, q]    <- khT as lhsT, qhT as rhs (c = 64)
  ctxT    [dim, q]  <- vh as lhsT, exp(scoresT) as rhs; a ones column fused
                       into vh makes row 64 of the accumulator the softmax
                       denominator for free
  out     [s, o]    <- ctxT as lhsT, WoT as rhs (c = 128, heads paired)

Attention processes head PAIRS with a q-range split (head h0 sweeps one half
of q while h1 sweeps the other, then they swap): the two heads' c=64 score
matmuls sit in adjacent row groups (partitions 0-63 / 64-127) so the PE
runs them as concurrent streams, and the PSUM footprint stays at 8 banks
(2x scores [128,1024] + 4x ctx [65,512]).

Softmax denominators roundtrip through DRAM twice: once spread over 64
partitions for a fast DVE reciprocal, once broadcast for the normalize.
Matmul operands are bf16 (fp32 PSUM accumulation); the attention scale
1/sqrt(64) is folded into Wq/bq on host.
"""

import numpy as np
import ml_dtypes

import concourse.bass as bass
import concourse.tile as tile
from concourse import bacc, mybir
from concourse.bass_utils import run_bass_kernel_spmd

B, S, SIZE, H, D = 2, 2048, 1024, 16, 64
NCORES = 8
HGROUPS = 4                # tensor-parallel head groups
H_LOC = H // HGROUPS       # 4 heads per core
D_LOC = H_LOC * D          # 256 projection dims per core
MT = D_LOC // 128          # 2 head-pairs per core
ET = SIZE // 128           # 8 contraction tiles for projections
ST = S // 128              # 16 sequence tiles of 128
QT = S // 512              # 4 sequence chunks of 512
KT = S // 128              # 16 key tiles

_NC = None


def build():
    global _NC
    if _NC is not None:
        return _NC
    f32, bf16 = mybir.dt.float32, mybir.dt.bfloat16
    Exp = mybir.ActivationFunctionType.Exp

    nc = bacc.Bacc("TRN2", target_bir_lowering=False, debug=False)
    qT_d = nc.dram_tensor("qT", [SIZE, S], bf16, kind="ExternalInput").ap()
    kT_d = nc.dram_tensor("kT", [SIZE, S], bf16, kind="ExternalInput").ap()
    vT_d = nc.dram_tensor("vT", [SIZE, S], bf16, kind="ExternalInput").ap()
    WqT_d = nc.dram_tensor("WqT", [SIZE, D_LOC], bf16, kind="ExternalInput").ap()
    WkT_d = nc.dram_tensor("WkT", [SIZE, D_LOC], bf16, kind="ExternalInput").ap()
    WvT_d = nc.dram_tensor("WvT", [SIZE, D_LOC], bf16, kind="ExternalInput").ap()
    WoT_d = nc.dram_tensor("WoT", [D_LOC, SIZE], bf16, kind="ExternalInput").ap()
    bq_d = nc.dram_tensor("bq", [D_LOC], f32, kind="ExternalInput").ap()
    bk_d = nc.dram_tensor("bk", [D_LOC], f32, kind="ExternalInput").ap()
    out_d = nc.dram_tensor("out", [S, SIZE], f32, kind="ExternalOutput").ap()

    qTt = qT_d.rearrange("(et p) s -> p et s", p=128)
    kTt = kT_d.rearrange("(et p) s -> p et s", p=128)
    vTt = vT_d.rearrange("(et p) s -> p et s", p=128)

    with tile.TileContext(nc) as tc:
        with tc.tile_pool(name="persist", bufs=1) as persist:
            wq_sb = persist.tile([128, ET, D_LOC], bf16)
            wk_sb = persist.tile([128, ET, D_LOC], bf16)
            wv_sb = persist.tile([128, ET, D_LOC], bf16)
            nc.sync.dma_start(wv_sb[:], WvT_d.rearrange("(et p) m -> p et m", p=128))
            nc.sync.dma_start(wq_sb[:], WqT_d.rearrange("(et p) m -> p et m", p=128))
            nc.sync.dma_start(wk_sb[:], WkT_d.rearrange("(et p) m -> p et m", p=128))
            wo_sb = persist.tile([128, MT, SIZE], bf16)
            nc.sync.dma_start(wo_sb[:], WoT_d.rearrange("(hp p) o -> p hp o", p=128))
            bq_sb = persist.tile([128, MT], f32)
            bk_sb = persist.tile([128, MT], f32)
            nc.sync.dma_start(bq_sb[:], bq_d.rearrange("(mt p) -> p mt", p=128))
            nc.sync.dma_start(bk_sb[:], bk_d.rearrange("(mt p) -> p mt", p=128))

            qh_sb = persist.tile([128, MT, S], bf16)   # [dim within pair, pair, s]
            kh_sb = persist.tile([128, MT, S], bf16)
            vh_sb = persist.tile([128, ST, H_LOC, D], bf16)  # [s%128, s//128, head, d]
            ctx_sb = persist.tile([128, MT, S], bf16)  # normalized ctxT, head pairs on partitions
            ones_f32 = persist.tile([128, 1], f32)
            nc.vector.memset(ones_f32[:], 1.0)
            ones_bf = persist.tile([128, 1], bf16)
            nc.vector.tensor_copy(ones_bf[:], ones_f32[:])

            # ---------- phase A: projections, one row-DMA per contraction tile ----------
            with (
                tc.tile_pool(name="xinV", bufs=3) as xinV,
                tc.tile_pool(name="psV", bufs=1, space="PSUM") as psV,
            ):
                # V first, in 2 passes of 8 sequence tiles (8 PSUM banks each)
                for vpass in range(2):
                    psv = [psV.tile([128, D_LOC], f32, tag=f"pv{i}", name=f"pv{i}")
                           for i in range(8)]
                    for et in range(ET):
                        vrow = xinV.tile([128, 1024], bf16, tag="vrow")
                        nc.sync.dma_start(
                            vrow[:], vTt[:, et, vpass * 1024:(vpass + 1) * 1024])
                        for i in range(8):
                            nc.tensor.matmul(
                                psv[i][:], vrow[:, i * 128:(i + 1) * 128], wv_sb[:, et, :],
                                start=(et == 0), stop=(et == ET - 1))
                    for i in range(8):
                        st = vpass * 8 + i
                        nc.vector.tensor_copy(
                            vh_sb[:, st, :, :],
                            psv[i].rearrange("p (h d) -> p h d", h=H_LOC))
            with (
                tc.tile_pool(name="xinQK", bufs=3) as xinQK,
                tc.tile_pool(name="psQK", bufs=1, space="PSUM") as psQK,
            ):
                for src, wsb, bsb, dst in (
                    (qTt, wq_sb, bq_sb, qh_sb),
                    (kTt, wk_sb, bk_sb, kh_sb),
                ):
                    pss = [psQK.tile([128, 512], f32, tag=f"pa{i}", name=f"pa{i}")
                           for i in range(8)]
                    for et in range(ET):
                        xrow = xinQK.tile([128, S], bf16, tag="xrow")
                        nc.sync.dma_start(xrow[:], src[:, et, :])
                        for nt in range(QT):
                            for mt in range(MT):
                                nc.tensor.matmul(
                                    pss[nt * MT + mt][:],
                                    wsb[:, et, mt * 128:(mt + 1) * 128],
                                    xrow[:, nt * 512:(nt + 1) * 512],
                                    start=(et == 0), stop=(et == ET - 1))
                    for nt in range(QT):
                        for mt in range(MT):
                            nc.vector.tensor_scalar_add(
                                dst[:, mt, nt * 512:(nt + 1) * 512],
                                pss[nt * MT + mt][:], bsb[:, mt:mt + 1])

            # ---------- phase B/C: attention, head pairs with q-split sweeps ----------
            with (
                tc.tile_pool(name="psS", bufs=1, space="PSUM") as psS,
                tc.tile_pool(name="psC", bufs=2, space="PSUM") as psC,
                tc.tile_pool(name="esb", bufs=4) as esb,
                tc.tile_pool(name="smalls", bufs=2) as smalls,
                tc.tile_pool(name="dscr", bufs=2, space="DRAM") as dscr,
            ):
                for pr in range(MT):
                    heads = (2 * pr, 2 * pr + 1)       # partition offsets 0, 64
                    for sw in range(2):
                        q0s = sw * 1024                # both heads sweep this q-half
                        # ctx col-packed: head 0 -> rows 0-63, head 1 -> rows
                        # 64-127 of one full-width accumulator per q-chunk
                        cpair = [psC.tile([128, 512], f32, tag=f"c{j}", name=f"c{j}")
                                 for j in range(2)]
                        # bf16 running sums are safe: only the 128-partition sum
                        # is used, so per-element rounding averages out; 4
                        # interleaved accumulators bound the sequential depth
                        eacc = [[smalls.tile([128, 1024], bf16, tag=f"eacc{hi}{a}",
                                             name=f"eacc{hi}{a}")
                                 for a in range(4)]
                                for hi in range(2)]
                        for kt in range(KT):
                            scs = [psS.tile([128, 1024], f32, tag=f"sc{hi}", name=f"sc{hi}")
                                   for hi in range(2)]
                            # interleave the two heads' matmuls: row groups 0-63 /
                            # 64-127 stream concurrently through the PE
                            for j in range(2):
                                for hi in range(2):
                                    po = hi * D
                                    q0 = q0s + j * 512
                                    nc.tensor.matmul(
                                        scs[hi][:, j * 512:(j + 1) * 512],
                                        kh_sb[po:po + D, pr, kt * 128:(kt + 1) * 128],
                                        qh_sb[po:po + D, pr, q0:q0 + 512],
                                        start=True, stop=True)
                            esbs = []
                            for hi in range(2):
                                e_sb = esb.tile([128, 1024], bf16, tag=f"e{hi}",
                                                name=f"e{hi}")
                                nc.scalar.activation(e_sb[:], scs[hi][:], Exp)
                                esbs.append(e_sb)
                            # softmax denominators: accumulate exp tiles on DVE
                            for hi in range(2):
                                a = kt % 4
                                if kt < 4:
                                    nc.vector.tensor_copy(eacc[hi][a][:], esbs[hi][:])
                                else:
                                    nc.vector.tensor_add(
                                        eacc[hi][a][:], eacc[hi][a][:], esbs[hi][:])
                            for j in range(2):
                                for hi in range(2):
                                    nc.tensor.matmul(
                                        cpair[j][hi * D:(hi + 1) * D, :],
                                        vh_sb[:, kt, heads[hi], :],
                                        esbs[hi][:, j * 512:(j + 1) * 512],
                                        start=(kt == 0), stop=(kt == KT - 1))
                        # evacuate accumulators (releases ctx PSUM banks), reduce
                        # eacc over partitions with tiny M=1 matmuls, reciprocal
                        # via the 64-partition DRAM spread, broadcast, multiply
                        cu = smalls.tile([128, 1024], f32, tag="cu")
                        for j in range(2):
                            nc.vector.tensor_copy(
                                cu[:, j * 512:(j + 1) * 512], cpair[j][:])
                        for hi in range(2):
                            nc.vector.tensor_add(
                                eacc[hi][0][:], eacc[hi][0][:], eacc[hi][1][:])
                            nc.vector.tensor_add(
                                eacc[hi][2][:], eacc[hi][2][:], eacc[hi][3][:])
                            nc.vector.tensor_add(
                                eacc[hi][0][:], eacc[hi][0][:], eacc[hi][2][:])
                        sums_sb = smalls.tile([1, 2048], f32, tag="sums")
                        for hi in range(2):
                            for j in range(2):
                                sp = psC.tile([1, 512], f32, tag=f"c{j}",
                                              name=f"sp{j}")
                                nc.tensor.matmul(
                                    sp[:], ones_bf[:],
                                    eacc[hi][0][:, j * 512:(j + 1) * 512],
                                    start=True, stop=True)
                                nc.vector.tensor_copy(
                                    sums_sb[:, hi * 1024 + j * 512:
                                            hi * 1024 + (j + 1) * 512], sp[:])
                        scr = dscr.tile([2048], f32, tag="scr")
                        nc.sync.dma_start(
                            scr[:].rearrange("(a b) -> a b", a=1), sums_sb[:])
                        spread = smalls.tile([64, 32], f32, tag="spread")
                        nc.sync.dma_start(
                            spread[:], scr[:].rearrange("(p j) -> p j", p=64))
                        spread_r = smalls.tile([64, 32], f32, tag="spreadr")
                        nc.vector.reciprocal(spread_r[:], spread[:])
                        scr2 = dscr.tile([2048], f32, tag="scr2")
                        nc.sync.dma_start(
                            scr2[:].rearrange("(p j) -> p j", p=64), spread_r[:])
                        brec = smalls.tile([128, 1024], f32, tag="brec")
                        for hi in range(2):
                            part = scr2[hi * 1024:(hi + 1) * 1024]
                            nc.sync.dma_start(
                                brec[hi * D:(hi + 1) * D, :],
                                bass.AP(tensor=part.tensor, offset=part.offset,
                                        ap=[[0, D]] + list(part.ap)))
                        for hi in range(2):
                            for j in range(2):
                                q0 = q0s + j * 512
                                rows = slice(hi * D, (hi + 1) * D)
                                nc.vector.tensor_mul(
                                    ctx_sb[rows, pr, q0:q0 + 512],
                                    cu[rows, j * 512:(j + 1) * 512],
                                    brec[rows, j * 512:(j + 1) * 512])

            # ---------- phase D: output projection (partial over local dims) ----------
            with (
                tc.tile_pool(name="psD", bufs=4, space="PSUM") as psD,
                tc.tile_pool(name="osb", bufs=4) as osb,
            ):
                for st in range(ST):
                    for ot in range(SIZE // 512):
                        pso = psD.tile([128, 512], f32, tag="po")
                        for hp in range(MT):
                            nc.tensor.matmul(
                                pso[:],
                                ctx_sb[:, hp, st * 128:(st + 1) * 128],
                                wo_sb[:, hp, ot * 512:(ot + 1) * 512],
                                start=(hp == 0), stop=(hp == MT - 1))
                        o_sb = osb.tile([128, 512], f32, tag="o")
                        if (st * 2 + ot) % 2 == 0:
                            nc.vector.tensor_copy(o_sb[:], pso[:])
                        else:
                            nc.scalar.copy(o_sb[:], pso[:])
                        nc.sync.dma_start(
                            out_d[st * 128:(st + 1) * 128, ot * 512:(ot + 1) * 512],
                            o_sb[:])

    nc.compile()
    _NC = nc
    return nc


def prepare_in_maps(inputs):
    q, k, v = inputs["q"], inputs["k"], inputs["v"]
    Wq, bq = inputs["Wq"], inputs["bq"]
    Wk, bk = inputs["Wk"], inputs["bk"]
    Wv = inputs["Wv"]
    Wo = inputs["Wo"]
    sc = np.float32(1.0 / np.sqrt(D))

    f32, bf = np.float32, ml_dtypes.bfloat16
    qT = [q[b].T.astype(bf) for b in range(B)]
    kT = [k[b].T.astype(bf) for b in range(B)]
    vT = [v[b].T.astype(bf) for b in range(B)]
    WqTs = (Wq.T * sc).astype(bf)   # scale folded into Wq
    WkT = Wk.T.astype(bf)
    WvT = Wv.T.astype(bf)
    WoT = Wo.T.astype(bf)           # [c, o]
    bqs = (bq * sc).astype(f32)

    in_maps = []
    for core in range(NCORES):
        b, hg = divmod(core, HGROUPS)
        sl = slice(hg * D_LOC, (hg + 1) * D_LOC)
        in_maps.append({
            "qT": qT[b], "kT": kT[b], "vT": vT[b],
            "WqT": np.ascontiguousarray(WqTs[:, sl]),
            "WkT": np.ascontiguousarray(WkT[:, sl]),
            "WvT": np.ascontiguousarray(WvT[:, sl]),
            "WoT": np.ascontiguousarray(WoT[sl, :]),
            "bq": np.ascontiguousarray(bqs[sl]),
            "bk": np.ascontiguousarray(bk[sl].astype(f32)),
        })
    return in_maps


def gather(results, inputs):
    # host epilogue: sum the 4 tensor-parallel partials per batch and add the
    # constant row bv @ Wo.T + bo (the value bias commutes through softmax)
    const = (inputs["bv"].astype(np.float64) @ inputs["Wo"].astype(np.float64).T
             + inputs["bo"].astype(np.float64)).astype(np.float32)
    full = np.empty((B, S, SIZE), np.float32)
    for b in range(B):
        acc = results[b * HGROUPS]["out"].astype(np.float32).copy()
        for hg in range(1, HGROUPS):
            acc += results[b * HGROUPS + hg]["out"]
        full[b] = acc + const[None, :]
    return full


def kernel(**inputs):
    nc = build()
    in_maps = prepare_in_maps(inputs)
    res = run_bass_kernel_spmd(nc, in_maps, core_ids=list(range(NCORES)), trace=False)
    return gather(res.results, inputs)
